# revision 1
# baseline (speedup 1.0000x reference)
"""Energy-function kernel v3: pipelined groups + split-precision matmul.

sims matmul runs as 3 half-precision products (xh*mh + xh*ml + xl*mh with
h=fp16, l=bf16 residual) at 1 cycle/row each - 0.75x the PE cycles of the
native fp32 path with ~5e-7 relative accuracy (validated on HW).

Pipelining: row-tiles go through in 4 groups of 4; per-group AllGathers
let merge + phase-2 of group g run in the matmul shadow of group g+1;
instruction emission is software-pipelined at tile granularity. The spill
is staged through a small SBUF ring per 512-chunk. geom + alpha tables are
emitted first so idle engines fill the matmul shadow.
"""

import os
import sys
from contextlib import ExitStack

import numpy as np

sys.path.insert(0, "/opt/trn_rl_repo")
sys.path.insert(0, "/opt/trn_rl_repo/concourse")

import concourse.tile as tile  # noqa: E402
from concourse import bacc, mybir  # noqa: E402
from concourse.bass_utils import run_bass_kernel_spmd  # noqa: E402

F32 = mybir.dt.float32
F16 = mybir.dt.float16
BF16 = mybir.dt.bfloat16
AF = mybir.ActivationFunctionType
ALU = mybir.AluOpType

B, N, D, K = 2048, 65536, 256, 32
NCORE = 8
NL = N // NCORE
RT = B // 128              # 16 row-tiles
GRP = 4                    # tiles per collective group
NG = RT // GRP             # 4 groups
ST = B // NCORE
CH = 512
SEG = 1024
NSEG = NL // SEG           # 8
W2 = 1024                  # phase-2 chunk width
BIGV = 40.0
EPS = 1e-4
NEG = -1.0e30

_CACHE = {}


def _build():
    SIM = os.environ.get("KSIM") == "1"
    nc = bacc.Bacc("TRN2", target_bir_lowering=False, debug=False,
                   num_devices=1 if SIM else NCORE)
    groups = [list(range(NCORE))]

    xTh_d = nc.dram_tensor("xTh", [D, B], F16, kind="ExternalInput").ap()
    xTl_d = nc.dram_tensor("xTl", [D, B], BF16, kind="ExternalInput").ap()
    xsT_d = nc.dram_tensor("xsT", [D, ST], F32, kind="ExternalInput").ap()
    muTh_d = nc.dram_tensor("muTh", [D, NL], F16, kind="ExternalInput").ap()
    muTl_d = nc.dram_tensor("muTl", [D, NL], BF16, kind="ExternalInput").ap()
    al_d = nc.dram_tensor("alpha_sh", [1, NL], F32, kind="ExternalInput").ap()
    W_d = nc.dram_tensor("Wm", [1, 3], F32, kind="ExternalInput").ap()
    b_d = nc.dram_tensor("bv", [1, 1], F32, kind="ExternalInput").ap()
    out_d = nc.dram_tensor("out", [B], F32, kind="ExternalOutput").ap()

    s_spill = [nc.dram_tensor(f"s_spill{i}", [128, NL], F32).ap()
               for i in range(RT)]
    V_loc = [nc.dram_tensor(f"V_loc{g}", [GRP, 128, NSEG * 8], F32).ap()
             for g in range(NG)]
    V_all = [nc.dram_tensor(f"V_all{g}", [NCORE, GRP, 128, NSEG * 8], F32).ap()
             for g in range(NG)]
    ar_in = nc.dram_tensor("ar_in", [128, RT + 1], F32).ap()
    ar_out = nc.dram_tensor("ar_out", [128, RT + 1], F32).ap()

    with tile.TileContext(nc) as tc, ExitStack() as ctx:
        const = ctx.enter_context(tc.tile_pool(name="const", bufs=1))
        psum = ctx.enter_context(tc.tile_pool(name="psum", bufs=6, space="PSUM"))
        psum_s = ctx.enter_context(tc.tile_pool(name="psum_s", bufs=2, space="PSUM"))

        # ---- constants ----
        ones1 = const.tile([1, 128], F32)
        nc.vector.memset(ones1[:], 1.0)
        ones_col = const.tile([128, 1], F32)
        nc.vector.memset(ones_col[:], 1.0)
        bias_negbig = const.tile([128, 1], F32)
        nc.vector.memset(bias_negbig[:], -BIGV)
        bias_1eps = const.tile([128, 1], F32)
        nc.vector.memset(bias_1eps[:], 1.0 + EPS)

        w_sb = const.tile([1, 3], F32)
        nc.sync.dma_start(w_sb[:], W_d)
        b_sb = const.tile([1, 1], F32)
        nc.sync.dma_start(b_sb[:], b_d)
        wb = const.tile([128, 4], F32)
        ps_wb = psum_s.tile([128, 4], F32, tag="ps_sm")
        nc.tensor.matmul(ps_wb[:, 0:3], ones1[:], w_sb[:], start=True, stop=True)
        nc.tensor.matmul(ps_wb[:, 3:4], ones1[:], b_sb[:], start=True, stop=True)
        nc.vector.tensor_copy(wb[:], ps_wb[:])

        xT0h = const.tile([128, B], F16)
        xT1h = const.tile([128, B], F16)
        xT0l = const.tile([128, B], BF16)
        xT1l = const.tile([128, B], BF16)
        nc.sync.dma_start(xT0h[:], xTh_d[0:128, :])
        nc.sync.dma_start(xT1h[:], xTh_d[128:256, :])
        nc.sync.dma_start(xT0l[:], xTl_d[0:128, :])
        nc.sync.dma_start(xT1l[:], xTl_d[128:256, :])

        tuv = const.tile([128, RT, 3], F32)
        S_loc = const.tile([128, RT], F32)
        argeom = const.tile([128, 1], F32)
        arout_sb = const.tile([128, RT + 1], F32)

        # ---- alpha tables (used by phase 2; fits in matmul shadow) ----
        ab = const.tile([128, NL], F32)
        cb2 = const.tile([128, NL], F32)
        setup_scope = ExitStack()
        scp = setup_scope.enter_context(tc.tile_pool(name="scp", bufs=1))
        al_sb = scp.tile([1, NL], F32)
        nc.sync.dma_start(al_sb[:], al_d)
        for j in range(NL // CH):
            ps_a = psum_s.tile([128, CH], F32, tag="ps_sm")
            nc.tensor.matmul(ps_a[:], ones1[:], al_sb[:, j * CH:(j + 1) * CH],
                             start=True, stop=True)
            nc.scalar.activation(ab[:, j * CH:(j + 1) * CH], ps_a[:],
                                 AF.Copy, scale=10.0)
        nc.vector.reciprocal(cb2[:], ab[:])
        nc.vector.tensor_scalar(cb2[:], cb2[:], BIGV, -1.0, ALU.mult, ALU.add)

        # ---- geom stripe (bf16); independent of phases ----
        gp = setup_scope.enter_context(tc.tile_pool(name="gp", bufs=2))
        gcp = setup_scope.enter_context(tc.tile_pool(name="gcp", bufs=1))
        xb0 = xT0h
        xb1 = xT1h
        xs0f = gcp.tile([128, ST], F32)
        xs1f = gcp.tile([128, ST], F32)
        nc.sync.dma_start(xs0f[:], xsT_d[0:128, :])
        nc.sync.dma_start(xs1f[:], xsT_d[128:256, :])
        xsb0 = gcp.tile([128, ST], F16)
        xsb1 = gcp.tile([128, ST], F16)
        nc.vector.tensor_copy(xsb0[:], xs0f[:])
        nc.vector.tensor_copy(xsb1[:], xs1f[:])

        nacc = (ST // 128) * (B // CH)
        gacc = gcp.tile([128, nacc], F32)
        dacc = gcp.tile([128, nacc], F32)
        for mt in range(ST // 128):
            for chn in range(B // CH):
                kcol = mt * (B // CH) + chn
                ps_g = psum_s.tile([128, CH], F32, tag="ps_sm")
                nc.tensor.matmul(ps_g[:], xsb0[:, mt * 128:(mt + 1) * 128],
                                 xb0[:, chn * CH:(chn + 1) * CH],
                                 start=True, stop=False)
                nc.tensor.matmul(ps_g[:], xsb1[:, mt * 128:(mt + 1) * 128],
                                 xb1[:, chn * CH:(chn + 1) * CH],
                                 start=False, stop=True)
                ucl = gp.tile([128, CH], F32, tag="ucl")
                nc.vector.tensor_scalar(ucl[:], ps_g[:], 0.999, None, ALU.min)
                lg = gp.tile([128, CH], F32, tag="lg")
                nc.scalar.activation(lg[:], ucl[:], AF.Ln, bias=bias_1eps[:, 0:1],
                                     scale=-1.0, accum_out=gacc[:, kcol:kcol + 1])
                dscr = gp.tile([128, CH], F32, tag="dscr")
                nc.vector.scalar_tensor_tensor(
                    out=dscr[:], in0=ucl[:], scalar=0.5, in1=lg[:],
                    op0=ALU.is_gt, op1=ALU.mult,
                    accum_out=dacc[:, kcol:kcol + 1])
        gs = gcp.tile([128, 2], F32)
        nc.vector.tensor_reduce(gs[:, 0:1], gacc[:], mybir.AxisListType.X, ALU.add)
        nc.vector.tensor_reduce(gs[:, 1:2], dacc[:], mybir.AxisListType.X, ALU.add)
        gvec = gcp.tile([128, 1], F32)
        nc.vector.tensor_sub(gvec[:], gs[:, 0:1], gs[:, 1:2])
        ps_sc = psum_s.tile([1, 1], F32, tag="ps_sm")
        nc.tensor.matmul(ps_sc[:], gvec[:], ones_col[:], start=True, stop=True)
        sc_sb = gcp.tile([1, 1], F32)
        nc.vector.tensor_copy(sc_sb[:], ps_sc[:])
        ps_bc = psum_s.tile([128, 1], F32, tag="ps_sm")
        nc.tensor.matmul(ps_bc[:], ones1[:], sc_sb[:], start=True, stop=True)
        nc.vector.tensor_copy(argeom[:], ps_bc[:])
        setup_scope.close()

        # ---- pipelined phases ----
        mupool = ctx.enter_context(tc.tile_pool(name="mupool", bufs=1))
        stage = ctx.enter_context(tc.tile_pool(name="stage", bufs=2))
        vpool = ctx.enter_context(tc.tile_pool(name="vpool", bufs=2))
        mp = ctx.enter_context(tc.tile_pool(name="mp", bufs=2))
        p2 = ctx.enter_context(tc.tile_pool(name="p2", bufs=3))
        p2b = ctx.enter_context(tc.tile_pool(name="p2b", bufs=2))

        muT0h = mupool.tile([128, NL], F16)
        muT1h = mupool.tile([128, NL], F16)
        muT0l = mupool.tile([128, NL], BF16)
        muT1l = mupool.tile([128, NL], BF16)
        nc.sync.dma_start(muT0h[:], muTh_d[0:128, :])
        nc.sync.dma_start(muT1h[:], muTh_d[128:256, :])
        nc.sync.dma_start(muT0l[:], muTl_d[0:128, :])
        nc.sync.dma_start(muT1l[:], muTl_d[128:256, :])

        NC64 = NSEG * 8

        def phase1_tile(i):
            vt = vpool.tile([128, NSEG, 8], F32, tag="vt")
            xsl = slice(i * 128, (i + 1) * 128)
            for g in range(NSEG):
                s_st = stage.tile([128, SEG], F32, tag="s_st")
                for h in range(SEG // CH):
                    lo = h * CH
                    msl = slice(g * SEG + lo, g * SEG + lo + CH)
                    pd = psum.tile([128, CH], F32, tag="ps1")
                    nc.tensor.matmul(pd[:], xT0h[:, xsl], muT0h[:, msl],
                                     start=True, stop=False)
                    nc.tensor.matmul(pd[:], xT0h[:, xsl], muT0l[:, msl],
                                     start=False, stop=False)
                    nc.tensor.matmul(pd[:], xT1h[:, xsl], muT1h[:, msl],
                                     start=False, stop=False)
                    nc.tensor.matmul(pd[:], xT1h[:, xsl], muT1l[:, msl],
                                     start=False, stop=False)
                    nc.tensor.matmul(pd[:], xT0l[:, xsl], muT0h[:, msl],
                                     start=False, stop=False)
                    nc.tensor.matmul(pd[:], xT1l[:, xsl], muT1h[:, msl],
                                     start=False, stop=True)
                    nc.scalar.copy(s_st[:, lo:lo + CH], pd[:])
                nc.vector.max(out=vt[:, g], in_=s_st[:])
                nc.sync.dma_start(
                    s_spill[i][:, g * SEG:(g + 1) * SEG], s_st[:])
            nc.sync.dma_start(V_loc[i // GRP][i % GRP],
                              vt[:].rearrange("p a b -> p (a b)"))

        def merge_tile(m):
            g = m // GRP
            vm = mp.tile([128, NCORE, NC64], F32, tag="vm")
            nc.sync.dma_start(
                vm[:], V_all[g][:, m - g * GRP].rearrange("a p c -> p a c"))
            vflat = vm[:].rearrange("p a b -> p (a b)")
            top32 = mp.tile([128, 32], F32, tag="top32")
            for r in range(4):
                nc.vector.max(out=top32[:, r * 8:(r + 1) * 8], in_=vflat)
                if r < 3:
                    nc.vector.match_replace(
                        out=vflat, in_to_replace=top32[:, r * 8:(r + 1) * 8],
                        in_values=vflat, imm_value=NEG)
            nc.vector.tensor_copy(tuv[:, m, 0:1], top32[:, 31:32])
            nc.vector.tensor_copy(tuv[:, m, 1:2], top32[:, 0:1])
            nc.vector.tensor_copy(tuv[:, m, 2:3], top32[:, 1:2])

        def phase2_tile(i):
            acc = p2b.tile([128, NL // W2], F32, tag="acc")
            for j2 in range(NL // W2):
                sl = slice(j2 * W2, (j2 + 1) * W2)
                s_in = p2.tile([128, W2], F32, tag="s_in")
                nc.sync.dma_start(s_in[:], s_spill[i][:, sl])
                t1 = p2.tile([128, W2], F32, tag="t1")
                nc.gpsimd.tensor_tensor(t1[:], s_in[:], cb2[:, sl], ALU.add)
                q = p2.tile([128, W2], F32, tag="q")
                nc.vector.scalar_tensor_tensor(
                    out=q[:], in0=s_in[:], scalar=tuv[:, i, 0:1], in1=t1[:],
                    op0=ALU.is_ge, op1=ALU.mult)
                nc.vector.tensor_tensor(q[:], q[:], ab[:, sl], ALU.mult)
                e_scr = p2.tile([128, W2], F32, tag="e_scr")
                nc.scalar.activation(e_scr[:], q[:], AF.Exp,
                                     bias=bias_negbig[:, 0:1],
                                     accum_out=acc[:, j2:j2 + 1])
            nc.vector.tensor_reduce(S_loc[:, i:i + 1], acc[:],
                                    mybir.AxisListType.X, ALU.add)

        # software-pipelined emission: ph1(i) interleaved with merge/ph2 of
        # the tile GRP behind, so each engine's priority queue alternates
        # between phase-1 and phase-2 work at tile granularity.
        for step in range(RT + GRP):
            if step < RT:
                phase1_tile(step)
            if step < RT and step % GRP == GRP - 1 and not SIM:
                g = step // GRP
                nc.gpsimd.collective_compute(
                    "AllGather", ALU.bypass, replica_groups=groups,
                    ins=[V_loc[g].opt()],
                    outs=[V_all[g].opt()])
            m = step - GRP
            if m >= 0:
                merge_tile(m)
                phase2_tile(m)

        # ---- allreduce S partials + geom ----
        fp = ctx.enter_context(tc.tile_pool(name="fp", bufs=2))
        nc.sync.dma_start(ar_in[:, 0:RT], S_loc[:])
        nc.sync.dma_start(ar_in[:, RT:RT + 1], argeom[:])
        if not SIM:
            nc.gpsimd.collective_compute(
                "AllReduce", ALU.add, replica_groups=groups,
                ins=[ar_in.opt()], outs=[ar_out.opt()])
        nc.sync.dma_start(arout_sb[:], ar_out)

        # ---- finale ----
        gterm = fp.tile([128, 1], F32, tag="gterm")
        nc.vector.tensor_scalar_mul(gterm[:], arout_sb[:, RT:RT + 1],
                                    -0.01 / (B * (B - 1.0)))
        out_r = out_d.rearrange("(t p one) -> t p one", p=128, one=1)
        for t in range(RT):
            lnS = fp.tile([128, 1], F32, tag="lnS")
            nc.scalar.activation(lnS[:], arout_sb[:, t:t + 1], AF.Ln)
            uu = tuv[:, t, 1:2]
            vv = tuv[:, t, 2:3]
            uvp = fp.tile([128, 1], F32, tag="uvp")
            nc.vector.tensor_tensor(uvp[:], uu, vv, ALU.mult)
            q1 = fp.tile([128, 1], F32, tag="q1")
            nc.vector.tensor_scalar(q1[:], uu, wb[:, 0:1], None, ALU.mult)
            q2 = fp.tile([128, 1], F32, tag="q2")
            nc.vector.scalar_tensor_tensor(out=q2[:], in0=vv, scalar=wb[:, 1:2],
                                           in1=q1[:], op0=ALU.mult, op1=ALU.add)
            q3 = fp.tile([128, 1], F32, tag="q3")
            nc.vector.scalar_tensor_tensor(out=q3[:], in0=uvp[:],
                                           scalar=wb[:, 2:3], in1=q2[:],
                                           op0=ALU.mult, op1=ALU.add)
            q4 = fp.tile([128, 1], F32, tag="q4")
            nc.vector.tensor_scalar(q4[:], q3[:], wb[:, 3:4], None, ALU.add)
            sg = fp.tile([128, 1], F32, tag="sg")
            nc.scalar.activation(sg[:], q4[:], AF.Sigmoid)
            e1 = fp.tile([128, 1], F32, tag="e1")
            nc.vector.scalar_tensor_tensor(out=e1[:], in0=sg[:], scalar=0.05,
                                           in1=lnS[:], op0=ALU.mult,
                                           op1=ALU.subtract)
            e2 = fp.tile([128, 1], F32, tag="e2")
            nc.vector.tensor_tensor(e2[:], e1[:], gterm[:], ALU.add)
            nc.sync.dma_start(out_r[t], e2[:])

    nc.compile()
    return nc


def kernel(**inputs):
    x = np.asarray(inputs["x"], dtype=np.float32)
    mu = np.asarray(inputs["mu"], dtype=np.float32)
    alpha = np.asarray(inputs["alpha"], dtype=np.float32)
    W = np.asarray(inputs["W"], dtype=np.float32).reshape(1, 3)
    b = np.asarray(inputs["b"], dtype=np.float32).reshape(1, 1)
    assert int(inputs["knn_k"]) == K

    if "nc" not in _CACHE:
        _CACHE["nc"] = _build()
    nc = _CACHE["nc"]

    import ml_dtypes
    xT = np.ascontiguousarray(x.T)
    xTh = xT.astype(np.float16)
    xTl = (xT - xTh.astype(np.float32)).astype(ml_dtypes.bfloat16)
    muT = np.ascontiguousarray(mu.T)          # [D, N]
    muTh = muT.astype(np.float16)
    muTl = (muT - muTh.astype(np.float32)).astype(ml_dtypes.bfloat16)
    in_maps = []
    for c in range(NCORE):
        csl = slice(c * NL, (c + 1) * NL)
        in_maps.append({
            "xTh": xTh,
            "xTl": xTl,
            "xsT": np.ascontiguousarray(x[c * ST:(c + 1) * ST].T),
            "muTh": np.ascontiguousarray(muTh[:, csl]),
            "muTl": np.ascontiguousarray(muTl[:, csl]),
            "alpha_sh": alpha[csl].reshape(1, NL),
            "Wm": W,
            "bv": b,
        })
    res = run_bass_kernel_spmd(nc, in_maps, core_ids=list(range(NCORE)))
    _CACHE["last_results"] = res
    return res.results[0]["out"].copy()

